# revision 19
# baseline (speedup 1.0000x reference)
"""Trainium2 Bass kernel for nn_DenseConcatBlocks (dense_cnn).

Strategy
--------
Data-parallel over batch: 16 images / 8 cores = 2 images per core, weights
replicated, one SPMD NEFF.

Per core, the 9x9 conv of every block is mapped onto the 128x128 PE array
with a Toeplitz "band" scheme that fills both array dimensions despite the
16-channel output:

  - M (stationary cols, 128) = 16 out-channels x 8 consecutive output rows
  - K (partitions, 128)      = 8 in-channels  x 16 input rows (out rows +-4)
  - N (moving dim, 268)      = both images' row pixels side by side (+pad)

The 9 kernel columns (dx) become 9 PSUM-accumulated matmuls reading the
same SBUF band tile at column offsets dx..dx+268; the 9 kernel rows (dy)
are absorbed into the banded stationary operand w1[o, c, g-s, dx].
Channels are chunked 8 at a time (c-chunks) and accumulated as well.

Feature activations live persistently in SBUF in "band layout": one
[128, 280] tile per (c-chunk j, band b); partition c_l*16+g holds channel
8j+c_l, image row 8b-4+g; columns are [8 zeros][img0 row][8 zeros]
[img1 row][8 zeros].  Each block's new channel (sigmoid output) is
scattered into its band slots by small DVE copies (each row lives in two
vertically-overlapping bands).

The 1x1 conv + sigmoid runs as a K=128 (s,o) -> M=8 (s) matmul with a
block-diagonal stationary operand, then two ScalarE sigmoids (bf16 copy
for the next block's features, f32 copy for the DRAM output).

Matmul operands are bf16 (PE streams 1 col/cycle; fp32 would be 4x
slower, fp32r needs rounded producers); PSUM accumulation is fp32 and all
bias/activation math is fp32.
"""

import sys

for _p in ("/opt/trn_rl_repo",):
    if _p not in sys.path:
        sys.path.insert(0, _p)

import numpy as np
import ml_dtypes

import concourse.bass as bass  # noqa: F401  (bass types used via tile/bacc)
import concourse.tile as tile
from concourse import bacc, mybir
from concourse.bass_utils import run_bass_kernel_spmd

H = W = 128
S = 8            # output rows per band
NB = H // S      # 16 bands
G = 16           # input rows per band (S + 8)
CPC = 8          # channels per c-chunk
WS = 280         # band tile width: 8z | 128 img0 | 8z | 128 img1 | 8z
N = 264          # matmul moving dim (slice dx+4 .. dx+268)
IMG0, IMG1 = 8, 144   # storage cols of img0/img1 w=0
P0, P1 = 0, 136       # psum cols of img0/img1 w=0
YW = 272              # y tile width (2*136)
NBLK = 50
N_CORES = 8
GB0 = 4          # bands per stationary-reuse group
DT = mybir.dt.bfloat16
NP_DT = ml_dtypes.bfloat16
F32 = mybir.dt.float32
Act = mybir.ActivationFunctionType


def _nch(c_in):
    return (c_in + CPC - 1) // CPC


def _pack_conv_lhsT(w1, nblk):
    """w1 [50,16,51,9,9] f32 -> [T,128,128] bf16 banded stationary tiles.

    Tile t for (blk, chunk j, dx): rows (c_l,g) = c_l*16+g, cols (s,o) =
    s*16+o, value w1[blk, o, 8j+c_l, g-s, dx] when 0 <= g-s <= 8 else 0.
    """
    g_idx, s_idx = np.meshgrid(np.arange(G), np.arange(S), indexing="ij")
    dy = g_idx - s_idx
    valid = (dy >= 0) & (dy <= 8)
    dyc = np.clip(dy, 0, 8)
    tiles = []
    for blk in range(nblk):
        c_in = blk + 1
        nch = _nch(c_in)
        out = np.zeros((nch, 9, 128, 128), np.float32)
        for j in range(nch):
            for c_l in range(CPC):
                c = CPC * j + c_l
                if c >= c_in:
                    continue
                wv = w1[blk, :, c]                      # [16, 9, 9]
                bl = wv[:, dyc, :] * valid[None, :, :, None]   # [16,G,S,9]
                bl = np.transpose(bl, (3, 1, 2, 0))     # [9, G, S, 16]
                out[j, :, c_l * 16:(c_l + 1) * 16, :] = bl.reshape(9, G, 128)
        tiles.append(out.reshape(nch * 9, 128, 128))
    return np.ascontiguousarray(np.concatenate(tiles, 0)).astype(NP_DT)


def _pack_aux(b1, w2, b2, nblk):
    # w1x1: [nblk, 128, 8]; row (s,o), col s' -> w2[blk,o] iff s == s'
    w1x1 = np.zeros((nblk, S, 16, S), np.float32)
    for s in range(S):
        w1x1[:, s, :, s] = w2[:nblk]
    w1x1 = w1x1.reshape(nblk, 128, S).astype(NP_DT)
    b1p = np.ascontiguousarray(np.tile(b1[:nblk], (1, S)), dtype=np.float32)
    b2p = np.ascontiguousarray(
        np.tile(b2[:nblk, None], (1, S)), dtype=np.float32)
    return w1x1, b1p, b2p


def _build(nblk, reps=1):
    nc = bacc.Bacc("TRN2", target_bir_lowering=False, debug=False)
    T = sum(_nch(i + 1) * 9 for i in range(nblk))
    x_in = nc.declare_dram_parameter("x_in", [2, H, W], F32, isOutput=False)
    wconv = nc.declare_dram_parameter("wconv", [T, 128, 128], DT, isOutput=False)
    w1x1 = nc.declare_dram_parameter("w1x1", [nblk, 128, S], DT, isOutput=False)
    b1p = nc.declare_dram_parameter("b1p", [nblk, 128], F32, isOutput=False)
    b2p = nc.declare_dram_parameter("b2p", [nblk, S], F32, isOutput=False)
    out = nc.declare_dram_parameter("out", [2, nblk, H, W], F32, isOutput=True)

    nch_tot = _nch(nblk)

    with tile.TileContext(nc) as tc:
        with (
            tc.tile_pool(name="bands", bufs=1) as bands_pool,
            tc.tile_pool(name="consts", bufs=1) as consts,
            tc.tile_pool(name="wpool", bufs=2) as wpool,
            tc.tile_pool(name="hpool", bufs=8) as hpool,
            tc.tile_pool(name="ypool", bufs=2) as ypool,
            tc.tile_pool(name="pscp", bufs=4, space="PSUM") as pscp,
            tc.tile_pool(name="psyp", bufs=4, space="PSUM") as psyp,
        ):
            bands = [
                bands_pool.tile(
                    [128, NB, WS], DT, name=f"band_{j}", tag=f"band_{j}")
                for j in range(nch_tot)
            ]
            x_sb = consts.tile([128, 2, W], F32, name="x_sb")
            nc.sync.dma_start(out=x_sb, in_=x_in.ap().rearrange("i h w -> h i w"))

            # x (channel 0) -> bf16, then DMA rows into chunk-0 band tiles.
            # (Engine ops need 32-aligned partition bases; DMA does not.)
            # Memset group 0 first so the x fill isn't stuck behind the
            # whole-chunk clears on the DVE FIFO.
            nc.vector.memset(bands[0][:, 0:GB0, :], 0.0)
            x_bf = consts.tile([128, 2, W], DT, name="x_bf")
            nc.vector.tensor_copy(out=x_bf, in_=x_sb)
            nc.vector.memset(bands[0][:, GB0:, :], 0.0)
            for b in range(NB):
                g0 = 4 if b == 0 else 0
                g1 = 12 if b == NB - 1 else 16
                r0 = 8 * b - 4 + g0
                dst = bands[0][g0:g1, b, IMG0:IMG0 + 272].rearrange(
                    "p (g w) -> p g w", w=136)[:, :, :W]
                nc.sync.dma_start(out=dst, in_=x_bf[r0:r0 + (g1 - g0)])
            w1x1_sb = consts.tile([128, nblk, S], DT, name="w1x1_sb")
            nc.sync.dma_start(
                out=w1x1_sb, in_=w1x1.ap().rearrange("t p m -> p t m"))
            b1_sb = consts.tile([128, nblk], F32, name="b1_sb")
            nc.sync.dma_start(out=b1_sb, in_=b1p.ap().rearrange("t p -> p t"))
            b2_sb = consts.tile([S, nblk], F32, name="b2_sb")
            nc.sync.dma_start(out=b2_sb, in_=b2p.ap().rearrange("t p -> p t"))
            for j in range(1, nch_tot):
                nc.vector.memset(bands[j], 0.0)

            toff = []
            acc = 0
            for i in range(nblk):
                toff.append(acc)
                acc += _nch(i + 1) * 9

            def load_w(b):
                n9b = _nch(b + 1) * 9
                w_t = wpool.tile(
                    [128, n9b, 128], DT, name=f"w_{b}", tag="wconv")
                nc.sync.dma_start(
                    out=w_t,
                    in_=wconv.ap()[toff[b]:toff[b] + n9b].rearrange(
                        "t p m -> p t m"))
                return w_t

            for rep in range(reps):
              wcur = load_w(0)
              for blk in range(nblk):
                c_in = blk + 1
                nch = _nch(c_in)
                n9 = nch * 9
                w_sb = wcur
                if blk + 1 < nblk:
                    wcur = load_w(blk + 1)
                y_bf = ypool.tile([S, NB, YW], DT, name=f"ybf_{blk}", tag="ybf")
                y_f32 = ypool.tile([S, NB, YW], F32, name=f"yf_{blk}", tag="yf32")
                hs = {}

                def finish(bb, blk=blk, y_bf=y_bf, y_f32=y_f32, hs=hs):
                    psy = psyp.tile(
                        [S, N], F32, name=f"psy_{blk}_{bb}", tag="psy")
                    nc.tensor.matmul(
                        psy, w1x1_sb[:, blk, :], hs.pop(bb),
                        start=True, stop=True)
                    nc.scalar.activation(
                        out=y_f32[:, bb, :N], in_=psy, func=Act.Sigmoid,
                        bias=b2_sb[:, blk:blk + 1], scale=1.0)
                    if blk + 1 < nblk:
                        nc.vector.tensor_copy(
                            out=y_bf[:, bb, :N], in_=y_f32[:, bb, :N])

                def scatter_group(b0, blk=blk, y_bf=y_bf):
                    # One DMA per overlap kind for GB bands: band tile dstb
                    # rows g 4..12 <- y band dstb (s 0..8), rows 12..16 <- y
                    # band dstb+1 (s 0..4), rows 0..4 <- y band dstb-1 (s 4..8)
                    c_new = blk + 1
                    base = (c_new % CPC) * 16
                    j2 = c_new // CPC

                    def cp(p0, np_, bd0, nb_, ys0, yb0):
                        for ic, pc in ((IMG0, P0), (IMG1, P1)):
                            dst = bands[j2][
                                base + p0:base + p0 + np_, bd0:bd0 + nb_,
                                ic:ic + W]
                            src = y_bf[
                                ys0:ys0 + np_, yb0:yb0 + nb_, pc:pc + W]
                            nc.sync.dma_start(out=dst, in_=src)

                    cp(4, 8, b0, GB0, 0, b0)
                    nh = min(b0 + GB0, NB - 1) - b0
                    if nh > 0:
                        cp(12, 4, b0, nh, 0, b0 + 1)
                    t0 = max(b0, 1)
                    nt = b0 + GB0 - t0
                    if nt > 0:
                        cp(0, 4, t0, nt, 4, t0 - 1)

                def out_group(b0, blk=blk, y_f32=y_f32):
                    for img, pc in ((0, P0), (1, P1)):
                        dst = out.ap()[img, blk].rearrange(
                            "(b s) w -> s b w", s=S)[:, b0:b0 + GB0]
                        nc.sync.dma_start(
                            out=dst, in_=y_f32[:, b0:b0 + GB0, pc:pc + W])

                # Bands in groups of GB share each stationary tile: the
                # (j, dx) loop is outside the band loop, so GB consecutive
                # matmuls reuse one LDWEIGHTS (amortizing the ~27ns/mm
                # weight-load the HW pays when every matmul swaps weights).
                GB = 4
                for gi in range(NB // GB):
                    g0 = GB * gi
                    pscs = [
                        pscp.tile([128, N], F32, name=f"psc_{blk}_{g0 + bb}",
                                  tag="psc")
                        for bb in range(GB)
                    ]
                    for j in range(nch):
                        for dx in range(9):
                            for bb in range(GB):
                                nc.tensor.matmul(
                                    pscs[bb],
                                    w_sb[:, j * 9 + dx, :],
                                    bands[j][:, g0 + bb, dx + 4:dx + 4 + N],
                                    start=(j == 0 and dx == 0),
                                    stop=(j == nch - 1 and dx == 8))
                    for bb in range(GB):
                        b = g0 + bb
                        h_t = hpool.tile(
                            [128, N], DT, name=f"h_{blk}_{b}", tag="h")
                        nc.scalar.activation(
                            out=h_t, in_=pscs[bb], func=Act.Relu,
                            bias=b1_sb[:, blk:blk + 1], scale=1.0)
                        hs[b] = h_t
                    if gi >= 1:
                        for bb in range(GB):
                            finish(GB * (gi - 1) + bb)
                        out_group(GB * (gi - 1))
                    if gi >= 2 and blk + 1 < nblk:
                        scatter_group(GB * (gi - 2))
                for bb in range(GB):
                    finish(NB - GB + bb)
                out_group(NB - GB)
                if blk + 1 < nblk:
                    scatter_group(NB - 2 * GB)
                    scatter_group(NB - GB)

    nc.compile()
    return nc


def _run(x, w1, b1, w2, b2, nblk=NBLK, trace=False):
    x = np.asarray(x, np.float32)
    wconv_np = _pack_conv_lhsT(np.asarray(w1, np.float32), nblk)
    w1x1_np, b1p_np, b2p_np = _pack_aux(
        np.asarray(b1, np.float32), np.asarray(w2, np.float32),
        np.asarray(b2, np.float32), nblk)
    nc = _build(nblk)
    in_maps = []
    for k in range(N_CORES):
        in_maps.append({
            "x_in": np.ascontiguousarray(x[2 * k:2 * k + 2, 0]),
            "wconv": wconv_np,
            "w1x1": w1x1_np,
            "b1p": b1p_np,
            "b2p": b2p_np,
        })
    res = run_bass_kernel_spmd(nc, in_maps, list(range(N_CORES)), trace=trace)
    full = np.concatenate([res.results[k]["out"] for k in range(N_CORES)], 0)
    return full, res


def kernel(**inputs):
    full, _ = _run(
        inputs["x"], inputs["w1"], inputs["b1"], inputs["w2"], inputs["b2"])
    return full.astype(np.float32)



# revision 24
# speedup vs baseline: 1.6399x; 1.6399x over previous
"""Trainium2 Bass kernel for nn_DenseConcatBlocks (dense_cnn).

Strategy
--------
Data-parallel over batch: 16 images / 8 cores = 2 images per core, weights
replicated, one SPMD NEFF.

Per core, the 9x9 conv of every block is mapped onto the 128x128 PE array
with a Toeplitz "band" scheme that fills both array dimensions despite the
16-channel output:

  - M (stationary cols, 128) = 16 out-channels x 8 consecutive output rows
  - K (partitions, 128)      = 8 in-channels  x 16 input rows (out rows +-4)
  - N (moving dim, 268)      = both images' row pixels side by side (+pad)

The 9 kernel columns (dx) become 9 PSUM-accumulated matmuls reading the
same SBUF band tile at column offsets dx..dx+268; the 9 kernel rows (dy)
are absorbed into the banded stationary operand w1[o, c, g-s, dx].
Channels are chunked 8 at a time (c-chunks) and accumulated as well.

Feature activations live persistently in SBUF in "band layout": one
[128, NB, 280] tile per c-chunk j; partition c_l*16+g holds channel
8j+c_l, image row 8b-4+g of band b; columns are [8 zeros][img0 row]
[8 zeros][img1 row][8 zeros].  Each block's new channel (sigmoid output)
is scattered into its band slots by 6 batched DMAs per 4-band group
(each row lives in two vertically-overlapping bands).

Bands are processed in groups of GB0=4 sharing each stationary (j, dx)
tile across 4 back-to-back matmuls into 4 live PSUM tiles: self-loading
matmuls that swap the stationary every issue pay ~27ns/mm of unhidden
weight-load on HW; >=4 reuses hide it (measured 3.93 -> 3.33ms).
Weights for block i+1 prefetch at the top of block i.

The 1x1 conv + sigmoid runs as a K=128 (s,o) -> M=8 (s) matmul with a
block-diagonal stationary operand, one ScalarE sigmoid producing the f32
DRAM copy, and a DVE tensor_copy deriving the bf16 feature copy.

Matmul operands are bf16 (PE streams 1 col/cycle; fp32 would be 4x
slower, fp32r needs rounded producers); PSUM accumulation is fp32 and all
bias/activation math is fp32.  All DMA triggers stay on the SP queue:
issuing them from the ACT queue measured +0.6ms, and splitting the
per-block output DMA into per-group strided DMAs measured +2ms.
"""

import sys

for _p in ("/opt/trn_rl_repo",):
    if _p not in sys.path:
        sys.path.insert(0, _p)

import numpy as np
import ml_dtypes

import concourse.bass as bass  # noqa: F401  (bass types used via tile/bacc)
import concourse.tile as tile
from concourse import bacc, mybir
from concourse.bass_utils import run_bass_kernel_spmd

H = W = 128
S = 8            # output rows per band
NB = H // S      # 16 bands
G = 16           # input rows per band (S + 8)
CPC = 8          # channels per c-chunk
WS = 280         # band tile width: 8z | 128 img0 | 8z | 128 img1 | 8z
N = 264          # matmul moving dim (slice dx+4 .. dx+268)
IMG0, IMG1 = 8, 144   # storage cols of img0/img1 w=0
P0, P1 = 0, 136       # psum cols of img0/img1 w=0
YW = 272              # y tile width (2*136)
NBLK = 50
N_CORES = 8
GB0 = 4          # bands per stationary-reuse group
DT = mybir.dt.bfloat16
NP_DT = ml_dtypes.bfloat16
F32 = mybir.dt.float32
Act = mybir.ActivationFunctionType


def _nch(c_in):
    return (c_in + CPC - 1) // CPC


def _pack_conv_lhsT(w1, nblk):
    """w1 [50,16,51,9,9] f32 -> [T,128,128] bf16 banded stationary tiles.

    Tile t for (blk, chunk j, dx): rows (c_l,g) = c_l*16+g, cols (s,o) =
    s*16+o, value w1[blk, o, 8j+c_l, g-s, dx] when 0 <= g-s <= 8 else 0.
    """
    g_idx, s_idx = np.meshgrid(np.arange(G), np.arange(S), indexing="ij")
    dy = g_idx - s_idx
    valid = (dy >= 0) & (dy <= 8)
    dyc = np.clip(dy, 0, 8)
    tiles = []
    for blk in range(nblk):
        c_in = blk + 1
        nch = _nch(c_in)
        out = np.zeros((nch, 9, 128, 128), np.float32)
        for j in range(nch):
            for c_l in range(CPC):
                c = CPC * j + c_l
                if c >= c_in:
                    continue
                wv = w1[blk, :, c]                      # [16, 9, 9]
                bl = wv[:, dyc, :] * valid[None, :, :, None]   # [16,G,S,9]
                bl = np.transpose(bl, (3, 1, 2, 0))     # [9, G, S, 16]
                out[j, :, c_l * 16:(c_l + 1) * 16, :] = bl.reshape(9, G, 128)
        tiles.append(out.reshape(nch * 9, 128, 128))
    return np.ascontiguousarray(np.concatenate(tiles, 0)).astype(NP_DT)


def _pack_aux(b1, w2, b2, nblk):
    # w1x1: [nblk, 128, 8]; row (s,o), col s' -> w2[blk,o] iff s == s'
    w1x1 = np.zeros((nblk, S, 16, S), np.float32)
    for s in range(S):
        w1x1[:, s, :, s] = w2[:nblk]
    w1x1 = w1x1.reshape(nblk, 128, S).astype(NP_DT)
    b1p = np.ascontiguousarray(np.tile(b1[:nblk], (1, S)), dtype=np.float32)
    b2p = np.ascontiguousarray(
        np.tile(b2[:nblk, None], (1, S)), dtype=np.float32)
    return w1x1, b1p, b2p


def _build(nblk, reps=1):
    nc = bacc.Bacc("TRN2", target_bir_lowering=False, debug=False)
    T = sum(_nch(i + 1) * 9 for i in range(nblk))
    x_in = nc.declare_dram_parameter("x_in", [2, H, W], F32, isOutput=False)
    wconv = nc.declare_dram_parameter("wconv", [T, 128, 128], DT, isOutput=False)
    w1x1 = nc.declare_dram_parameter("w1x1", [nblk, 128, S], DT, isOutput=False)
    b1p = nc.declare_dram_parameter("b1p", [nblk, 128], F32, isOutput=False)
    b2p = nc.declare_dram_parameter("b2p", [nblk, S], F32, isOutput=False)
    out = nc.declare_dram_parameter("out", [2, nblk, H, W], F32, isOutput=True)

    nch_tot = _nch(nblk)

    with tile.TileContext(nc) as tc:
        with (
            tc.tile_pool(name="bands", bufs=1) as bands_pool,
            tc.tile_pool(name="consts", bufs=1) as consts,
            tc.tile_pool(name="wpool", bufs=2) as wpool,
            tc.tile_pool(name="hpool", bufs=8) as hpool,
            tc.tile_pool(name="ypool", bufs=2) as ypool,
            tc.tile_pool(name="pscp", bufs=4, space="PSUM") as pscp,
            tc.tile_pool(name="psyp", bufs=4, space="PSUM") as psyp,
        ):
            bands = [
                bands_pool.tile(
                    [128, NB, WS], DT, name=f"band_{j}", tag=f"band_{j}")
                for j in range(nch_tot)
            ]
            nc.vector.memset(bands[0], 0.0)
            x_sb = consts.tile([128, 2, W], F32, name="x_sb")
            nc.sync.dma_start(out=x_sb, in_=x_in.ap().rearrange("i h w -> h i w"))
            w1x1_sb = consts.tile([128, nblk, S], DT, name="w1x1_sb")
            nc.sync.dma_start(
                out=w1x1_sb, in_=w1x1.ap().rearrange("t p m -> p t m"))
            b1_sb = consts.tile([128, nblk], F32, name="b1_sb")
            nc.sync.dma_start(out=b1_sb, in_=b1p.ap().rearrange("t p -> p t"))
            b2_sb = consts.tile([S, nblk], F32, name="b2_sb")
            nc.sync.dma_start(out=b2_sb, in_=b2p.ap().rearrange("t p -> p t"))

            # x (channel 0) -> bf16, then DMA rows into chunk-0 band tiles.
            # (Engine ops need 32-aligned partition bases; DMA does not.)
            x_bf = consts.tile([128, 2, W], DT, name="x_bf")
            nc.vector.tensor_copy(out=x_bf, in_=x_sb)
            for b in range(NB):
                g0 = 4 if b == 0 else 0
                g1 = 12 if b == NB - 1 else 16
                r0 = 8 * b - 4 + g0
                dst = bands[0][g0:g1, b, IMG0:IMG0 + 272].rearrange(
                    "p (g w) -> p g w", w=136)[:, :, :W]
                nc.sync.dma_start(out=dst, in_=x_bf[r0:r0 + (g1 - g0)])
            for j in range(1, nch_tot):
                nc.vector.memset(bands[j], 0.0)

            toff = []
            acc = 0
            for i in range(nblk):
                toff.append(acc)
                acc += _nch(i + 1) * 9

            def load_w(b):
                n9b = _nch(b + 1) * 9
                w_t = wpool.tile(
                    [128, n9b, 128], DT, name=f"w_{b}", tag="wconv")
                nc.sync.dma_start(
                    out=w_t,
                    in_=wconv.ap()[toff[b]:toff[b] + n9b].rearrange(
                        "t p m -> p t m"))
                return w_t

            for rep in range(reps):
              wcur = load_w(0)
              for blk in range(nblk):
                c_in = blk + 1
                nch = _nch(c_in)
                n9 = nch * 9
                w_sb = wcur
                if blk + 1 < nblk:
                    wcur = load_w(blk + 1)
                y_bf = ypool.tile([S, NB, YW], DT, name=f"ybf_{blk}", tag="ybf")
                y_f32 = ypool.tile([S, NB, YW], F32, name=f"yf_{blk}", tag="yf32")
                hs = {}

                def finish(bb, blk=blk, y_bf=y_bf, y_f32=y_f32, hs=hs):
                    psy = psyp.tile(
                        [S, N], F32, name=f"psy_{blk}_{bb}", tag="psy")
                    nc.tensor.matmul(
                        psy, w1x1_sb[:, blk, :], hs.pop(bb),
                        start=True, stop=True)
                    nc.scalar.activation(
                        out=y_f32[:, bb, :N], in_=psy, func=Act.Sigmoid,
                        bias=b2_sb[:, blk:blk + 1], scale=1.0)
                    if blk + 1 < nblk:
                        nc.vector.tensor_copy(
                            out=y_bf[:, bb, :N], in_=y_f32[:, bb, :N])

                def scatter_group(b0, blk=blk, y_bf=y_bf):
                    # One DMA per overlap kind for GB bands: band tile dstb
                    # rows g 4..12 <- y band dstb (s 0..8), rows 12..16 <- y
                    # band dstb+1 (s 0..4), rows 0..4 <- y band dstb-1 (s 4..8)
                    c_new = blk + 1
                    base = (c_new % CPC) * 16
                    j2 = c_new // CPC

                    def cp(p0, np_, bd0, nb_, ys0, yb0):
                        for ic, pc in ((IMG0, P0), (IMG1, P1)):
                            dst = bands[j2][
                                base + p0:base + p0 + np_, bd0:bd0 + nb_,
                                ic:ic + W]
                            src = y_bf[
                                ys0:ys0 + np_, yb0:yb0 + nb_, pc:pc + W]
                            nc.sync.dma_start(out=dst, in_=src)

                    cp(4, 8, b0, GB0, 0, b0)
                    nh = min(b0 + GB0, NB - 1) - b0
                    if nh > 0:
                        cp(12, 4, b0, nh, 0, b0 + 1)
                    t0 = max(b0, 1)
                    nt = b0 + GB0 - t0
                    if nt > 0:
                        cp(0, 4, t0, nt, 4, t0 - 1)

                # Bands in groups of GB share each stationary tile: the
                # (j, dx) loop is outside the band loop, so GB consecutive
                # matmuls reuse one LDWEIGHTS (amortizing the ~27ns/mm
                # weight-load the HW pays when every matmul swaps weights).
                GB = 4
                for gi in range(NB // GB):
                    g0 = GB * gi
                    pscs = [
                        pscp.tile([128, N], F32, name=f"psc_{blk}_{g0 + bb}",
                                  tag="psc")
                        for bb in range(GB)
                    ]
                    for j in range(nch):
                        for dx in range(9):
                            for bb in range(GB):
                                nc.tensor.matmul(
                                    pscs[bb],
                                    w_sb[:, j * 9 + dx, :],
                                    bands[j][:, g0 + bb, dx + 4:dx + 4 + N],
                                    start=(j == 0 and dx == 0),
                                    stop=(j == nch - 1 and dx == 8))
                    for bb in range(GB):
                        b = g0 + bb
                        h_t = hpool.tile(
                            [128, N], DT, name=f"h_{blk}_{b}", tag="h")
                        nc.scalar.activation(
                            out=h_t, in_=pscs[bb], func=Act.Relu,
                            bias=b1_sb[:, blk:blk + 1], scale=1.0)
                        hs[b] = h_t
                    if gi >= 1:
                        for bb in range(GB):
                            finish(GB * (gi - 1) + bb)
                    if gi >= 2 and blk + 1 < nblk:
                        scatter_group(GB * (gi - 2))
                for bb in range(GB):
                    finish(NB - GB + bb)
                if blk + 1 < nblk:
                    scatter_group(NB - 2 * GB)
                    scatter_group(NB - GB)
                for img, pc in ((0, P0), (1, P1)):
                    dst = out.ap()[img, blk].rearrange("(b s) w -> s b w", s=S)
                    nc.sync.dma_start(
                        out=dst, in_=y_f32[:, :, pc:pc + W])

    nc.compile()
    return nc


def _run(x, w1, b1, w2, b2, nblk=NBLK, trace=False):
    x = np.asarray(x, np.float32)
    wconv_np = _pack_conv_lhsT(np.asarray(w1, np.float32), nblk)
    w1x1_np, b1p_np, b2p_np = _pack_aux(
        np.asarray(b1, np.float32), np.asarray(w2, np.float32),
        np.asarray(b2, np.float32), nblk)
    nc = _build(nblk)
    in_maps = []
    for k in range(N_CORES):
        in_maps.append({
            "x_in": np.ascontiguousarray(x[2 * k:2 * k + 2, 0]),
            "wconv": wconv_np,
            "w1x1": w1x1_np,
            "b1p": b1p_np,
            "b2p": b2p_np,
        })
    res = run_bass_kernel_spmd(nc, in_maps, list(range(N_CORES)), trace=trace)
    full = np.concatenate([res.results[k]["out"] for k in range(N_CORES)], 0)
    return full, res


def kernel(**inputs):
    full, _ = _run(
        inputs["x"], inputs["w1"], inputs["b1"], inputs["w2"], inputs["b2"])
    return full.astype(np.float32)

